# revision 24
# baseline (speedup 1.0000x reference)
"""DeepseekV2 MLA attention (prefill, causal) on 8 trn2 NeuronCores — v2.

Math: non-absorbed form (optimal for prefill):
    k_nope = ckv @ w_uk,  v = ckv @ w_uv          (per head)
    scores = [q_nope;q_pe] . [k_nope;k_pe]        (d = 192)
    out    = softmax(scores) @ v ;  y = concat_h(out) @ w_o

Sharding & wire plan (replaces v1's AllGather(q_a)+ReduceScatter(y)):
  - Projections are sequence-sharded (256 rows/core); attention is
    head-sharded (2 heads/core); y is sequence-sharded again.
  - AllGather moves only ckv+k_pe (576x256 bf16 = 0.3 MB/core).
  - q is projected for ALL 16 heads on the row-owning core, then
    AllToAll'd to the head-owning cores in two waves (nope 1 MB, rope
    0.5 MB bf16, mesh one-hop) so the wire overlaps the rope compute.
  - o is AllToAll'd back to row owners (1 MB bf16); each core computes
    its own 256-row slice of y with the full (bf16) w_o.  No reduce.
  - RMSNorm weights are folded into the downstream matmuls on the host;
    x is pre-transposed and laid out partition-major on the host (big
    contiguous DMA lines).  All PE stationary operands are bf16 so the
    compiler's fast-weight-load kicks in; PSUM stays f32.
  - DMA triggers are spread over the sync/scalar/gpsimd queues and
    ordered so bulk weights (w_qb, w_o) cannot starve the critical
    path; w_o streams during attention's DMA-idle window.  The softmax
    divide of chunk qc issues under chunk qc+1's first score matmuls;
    1/sum comes from exp(-ln(sum)) on the scalar engine.
"""
import sys

sys.path.insert(0, "/opt/trn_rl_repo")

import numpy as np
import ml_dtypes

import concourse.bass as bass
from concourse import bacc
import concourse.mybir as mybir
import concourse.tile as tile
from concourse.bass_utils import run_bass_kernel_spmd

F32 = mybir.dt.float32
BF16 = mybir.dt.bfloat16
AF = mybir.ActivationFunctionType
BFNP = ml_dtypes.bfloat16

B, S, E, H = 1, 2048, 2048, 16
DN, DR, DV, R, QLR = 128, 64, 128, 512, 1536
EPS = 1e-6
NCORES = 8
SL = S // NCORES          # 256 sequence rows per core
HPC = H // NCORES         # 2 heads per core
SM_SCALE = (DN + DR) ** -0.5
NEG = -1e30
ROPE_BASE = 10000.0

EC = E // 128             # 16 contraction chunks over E
QC = QLR // 128           # 12 chunks over QLR
NQC = S // 512            # 4 query column chunks
NKT = S // 128            # 16 key tiles
AGR = R + DR              # 576 rows in the allgather payload
AQB = 2 * DN + 2 * DR     # 384 rows per a2a-q shard (2 heads nope + pe)
AQR = NCORES * AQB        # 3072
AOB = HPC * DV            # 256 rows per a2a-o shard
AOR = NCORES * AOB        # 2048


def _rope_rm():
    """Row-major cos/sin tables [S, DR] (fp64 -> f32)."""
    inv_freq = 1.0 / (ROPE_BASE ** (np.arange(0, DR, 2, dtype=np.float64) / DR))
    ang = np.arange(S, dtype=np.float64)[:, None] * inv_freq[None, :]
    cos = np.concatenate([np.cos(ang), np.cos(ang)], -1).astype(np.float32)
    sin = np.concatenate([np.sin(ang), np.sin(ang)], -1).astype(np.float32)
    return cos, sin  # [S, 64]


def _masks():
    # scoresT tile [k 128 | q 512]; m = kt - 4*qc; valid iff q >= k
    ii = np.arange(128)[:, None]
    jj = np.arange(512)[None, :]
    return np.stack(
        [np.where(jj - ii - 128 * m >= 0, 0.0, NEG).astype(BFNP) for m in range(4)]
    )  # [4,128,512]


def _build(skip_collectives=False):
    nc = bacc.Bacc(None, num_devices=NCORES)

    # partition-major [128, ...] layouts (host-prepared) so each load is one
    # DMA with multi-KB contiguous lines per partition, not 512B descriptors
    xT = nc.dram_tensor("xT_pm", [128, EC * SL], BF16, kind="ExternalInput")
    w_qa = nc.dram_tensor("w_qa", [E, QLR], BF16, kind="ExternalInput")
    w_kva = nc.dram_tensor("w_kva_pm", [128, EC * (R + DR)], BF16,
                           kind="ExternalInput")
    w_qb = nc.dram_tensor("w_qb", [QLR, H * (DN + DR)], BF16, kind="ExternalInput")
    w_uk = nc.dram_tensor("w_uk_pm", [128, 4 * HPC * DN], BF16,
                          kind="ExternalInput")
    w_uv = nc.dram_tensor("w_uv_pm", [128, 4 * HPC * DV], BF16,
                          kind="ExternalInput")
    w_o = nc.dram_tensor("w_o_f", [H * DV, E], BF16, kind="ExternalInput")
    cosq = nc.dram_tensor("cosq_pm", [128, 2 * DR], BF16, kind="ExternalInput")
    sinq = nc.dram_tensor("sinq_pm", [128, 2 * DR], BF16, kind="ExternalInput")
    cos16 = nc.dram_tensor("cos16_pm", [128, 2 * H * DR], BF16,
                           kind="ExternalInput")
    sin16 = nc.dram_tensor("sin16_pm", [128, 2 * H * DR], BF16,
                           kind="ExternalInput")
    masks = nc.dram_tensor("masks_pm", [128, 4 * 512], BF16, kind="ExternalInput")
    ones_in = nc.dram_tensor("ones_in", [128, 128], BF16, kind="ExternalInput")
    ident_in = nc.dram_tensor("ident_in", [128, 128], BF16, kind="ExternalInput")
    y_sl = nc.dram_tensor("y_sl", [SL, E], F32, kind="ExternalOutput")

    ag_in = nc.dram_tensor("ag_in", [AGR, SL], BF16)
    ag_out = nc.dram_tensor("ag_out", [NCORES * AGR, SL], BF16, addr_space="Shared")
    aqa_in = nc.dram_tensor("aqa_in", [NCORES * 2 * DN, SL], BF16)
    aqa_out = nc.dram_tensor("aqa_out", [NCORES * 2 * DN, SL], BF16)
    aqb_in = nc.dram_tensor("aqb_in", [NCORES * 2 * DR, SL], BF16)
    aqb_out = nc.dram_tensor("aqb_out", [NCORES * 2 * DR, SL], BF16)
    ao_in = nc.dram_tensor("ao_in", [AOR, SL], BF16)
    ao_out = nc.dram_tensor("ao_out", [AOR, SL], BF16)

    def cc(kind, op, in_t, out_t):
        if skip_collectives:
            if kind == "AllGather":
                for c in range(NCORES):
                    rows = in_t.shape[0]
                    nc.gpsimd.dma_start(
                        out=out_t[c * rows:(c + 1) * rows, :], in_=in_t[:, :])
            else:
                nc.gpsimd.dma_start(out=out_t[:, :], in_=in_t[:, :])
        else:
            nc.gpsimd.collective_compute(
                kind, op, replica_groups=[list(range(NCORES))],
                ins=[in_t[:, :].opt()], outs=[out_t[:, :].opt()])

    with tile.TileContext(nc) as tc:
        with tc.tile_pool(name="consts", bufs=1) as cp:
            ones_sb = cp.tile([128, 128], BF16)
            nc.sync.dma_start(out=ones_sb, in_=ones_in[:, :])
            ident_sb = cp.tile([128, 128], BF16)
            nc.sync.dma_start(out=ident_sb, in_=ident_in[:, :])
            eps_sb = cp.tile([128, 1], F32)
            nc.vector.memset(eps_sb[:], EPS)

            # ------------- sender phase: projections on my 256 rows -------------
            with tc.tile_pool(name="snd", bufs=1) as sp, \
                 tc.tile_pool(name="psA", bufs=1, space="PSUM") as psA:
                xT_sb = sp.tile([128, EC, SL], BF16, tag="xT")
                xT_v = xT.rearrange("p (ec s) -> p ec s", ec=EC)
                nc.sync.dma_start(out=xT_sb[:, 0:8, :], in_=xT_v[:, 0:8, :])
                nc.scalar.dma_start(out=xT_sb[:, 8:EC, :], in_=xT_v[:, 8:EC, :])

                # --- ckv + k_pe (row-major), rmsnorm, rope; ship to AllGather ---
                ps_ckv = [psA.tile([128, 512], F32, tag=f"a{r}", name=f"psckv{r}", bufs=1)
                          for r in range(2)]
                ps_pe = [psA.tile([128, DR], F32, tag=f"a{2 + r}", name=f"pspe{r}", bufs=1)
                         for r in range(2)]
                wkva_v = w_kva.rearrange("p (ec m) -> p ec m", ec=EC)
                for e in range(EC):
                    wk = sp.tile([128, R + DR], BF16, tag="wkva", bufs=6)
                    nc.sync.dma_start(out=wk, in_=wkva_v[:, e, :])
                    for r in range(2):
                        rs = slice(r * 128, (r + 1) * 128)
                        nc.tensor.matmul(ps_ckv[r][:], xT_sb[:, e, rs], wk[:, 0:R],
                                         start=(e == 0), stop=(e == EC - 1))
                        nc.tensor.matmul(ps_pe[r][:], xT_sb[:, e, rs], wk[:, R:],
                                         start=(e == 0), stop=(e == EC - 1))
                cosq_sb = sp.tile([128, 2, DR], BF16, tag="cosq")
                nc.sync.dma_start(
                    out=cosq_sb, in_=cosq.rearrange("p (r d) -> p r d", r=2))
                sinq_sb = sp.tile([128, 2, DR], BF16, tag="sinq")
                nc.sync.dma_start(
                    out=sinq_sb, in_=sinq.rearrange("p (r d) -> p r d", r=2))
                ckv_n = sp.tile([128, 2, R], BF16, tag="ckv_n")
                kpe_o = sp.tile([128, 2, DR], BF16, tag="kpe_o")
                sq_junk = sp.tile([128, 512], F32, tag="sq_junk", bufs=1)
                for r in range(2):
                    ssq = sp.tile([128, 1], F32, tag="ssq", bufs=2)
                    nc.scalar.activation(out=sq_junk, in_=ps_ckv[r][:],
                                         func=AF.Square, accum_out=ssq[:])
                    rstd = sp.tile([128, 1], F32, tag="rstd", bufs=2)
                    nc.scalar.activation(out=rstd, in_=ssq[:], func=AF.Sqrt,
                                         scale=1.0 / R, bias=eps_sb[:])
                    nc.vector.reciprocal(rstd[:], rstd[:])
                    nc.vector.tensor_scalar_mul(ckv_n[:, r, :], ps_ckv[r][:], rstd[:])
                    # rope on k_pe (row-major: shuffle along free dim)
                    rot = sp.tile([128, DR], F32, tag="rot", bufs=2)
                    nc.vector.tensor_scalar_mul(rot[:, 0:32], ps_pe[r][:, 32:64], -1.0)
                    nc.vector.tensor_copy(rot[:, 32:64], ps_pe[r][:, 0:32])
                    t1 = sp.tile([128, DR], F32, tag="t1", bufs=2)
                    nc.vector.tensor_mul(t1[:], ps_pe[r][:], cosq_sb[:, r, :])
                    t2 = sp.tile([128, DR], F32, tag="t2", bufs=2)
                    nc.vector.tensor_mul(t2[:], rot[:], sinq_sb[:, r, :])
                    nc.vector.tensor_add(kpe_o[:, r, :], t1[:], t2[:])
                # transpose [2*128, 576] -> [576, 256] and ship
                ag_stage = sp.tile([128, 4, SL], BF16, tag="ag_stage")
                agpe_stage = sp.tile([64, SL], BF16, tag="agpe_stage")
                for r in range(2):
                    for cch in range(4):
                        pt = psA.tile([128, 128], BF16, tag="ptA", bufs=2)
                        nc.tensor.transpose(
                            pt[:], ckv_n[:, r, cch * 128:(cch + 1) * 128], ident_sb[:])
                        nc.vector.tensor_copy(
                            ag_stage[:, cch, r * 128:(r + 1) * 128], pt[:])
                    ptp = psA.tile([64, 128], BF16, tag="ptA", bufs=2)
                    nc.tensor.transpose(ptp[:], kpe_o[:, r, :], ident_sb[:])
                    nc.vector.tensor_copy(agpe_stage[:, r * 128:(r + 1) * 128], ptp[:])
                nc.gpsimd.dma_start(
                    out=ag_in[0:R, :].rearrange("(rc p) s -> p rc s", p=128),
                    in_=ag_stage[:])
                nc.gpsimd.dma_start(out=ag_in[R:AGR, :], in_=agpe_stage[:])
                cc("AllGather", mybir.AluOpType.bypass, ag_in, ag_out)

                wqb_v = w_qb.rearrange("(qc p) m -> p qc m", p=128)

                # w_qb first nope quarter loads early on the scalar queue
                # (fires after the ckv rmsnorm ACTs).
                wqb_sb = sp.tile([128, QC, 2 * 512], BF16, tag="wqb",
                                 name="wqb_sb", bufs=1)
                for ch in range(QC):
                    nc.scalar.dma_start(
                        out=wqb_sb[:, ch, :], in_=wqb_v[:, ch, 0:1024])

                # --- q_a (row-major) + rmsnorm ---
                psq = [psA.tile([128, 512], F32, tag=f"a{i}", name=f"psq{i}", bufs=1)
                       for i in range(6)]
                wqa_v = w_qa.rearrange("(ec p) m -> p ec m", p=128)
                for e in range(EC):
                    wq = sp.tile([128, QLR], BF16, tag="wqa", bufs=3)
                    nc.gpsimd.dma_start(out=wq, in_=wqa_v[:, e, :])
                    for r in range(2):
                        rs = slice(r * 128, (r + 1) * 128)
                        for j in range(3):
                            nc.tensor.matmul(
                                psq[3 * r + j][:], xT_sb[:, e, rs],
                                wq[:, j * 512:(j + 1) * 512],
                                start=(e == 0), stop=(e == EC - 1))
                qa_n = sp.tile([128, 2, QLR], BF16, tag="qa_n")
                for r in range(2):
                    ssqs = []
                    for j in range(3):
                        ssq = sp.tile([128, 1], F32, tag=f"ssqq{j}", bufs=2)
                        nc.scalar.activation(out=sq_junk, in_=psq[3 * r + j][:],
                                             func=AF.Square, accum_out=ssq[:])
                        ssqs.append(ssq)
                    nc.vector.tensor_add(ssqs[0][:], ssqs[0][:], ssqs[1][:])
                    nc.vector.tensor_add(ssqs[0][:], ssqs[0][:], ssqs[2][:])
                    rstd = sp.tile([128, 1], F32, tag="rstdq", bufs=2)
                    nc.scalar.activation(out=rstd, in_=ssqs[0][:], func=AF.Sqrt,
                                         scale=1.0 / QLR, bias=eps_sb[:])
                    nc.vector.reciprocal(rstd[:], rstd[:])
                    for j in range(3):
                        nc.vector.tensor_scalar_mul(
                            qa_n[:, r, j * 512:(j + 1) * 512], psq[3 * r + j][:],
                            rstd[:])
                # transpose q_a -> q_aT [1536, 256] (stationary for q_b)
                qaT = sp.tile([128, QC, SL], BF16, tag="qaT")
                for r in range(2):
                    for ch in range(QC):
                        pt = psA.tile([128, 128], BF16, tag="ptA", bufs=2)
                        nc.tensor.transpose(
                            pt[:], qa_n[:, r, ch * 128:(ch + 1) * 128], ident_sb[:])
                        nc.vector.tensor_copy(qaT[:, ch, r * 128:(r + 1) * 128], pt[:])

                # --- q = q_aT.T @ w_qb (row-major out), rope, transpose, ship ---
                qn_sb = sp.tile([128, 2, H * DN], BF16, tag="qn_sb")
                qpe_raw = sp.tile([128, 2, H * DR], BF16, tag="qpe_raw")
                c16_sb = sp.tile([128, 2, H * DR], BF16, tag="c16")
                nc.sync.dma_start(
                    out=c16_sb, in_=cos16.rearrange("p (r d) -> p r d", r=2))
                s16_sb = sp.tile([128, 2, H * DR], BF16, tag="s16")
                nc.sync.dma_start(
                    out=s16_sb, in_=sin16.rearrange("p (r d) -> p r d", r=2))
                qpe_rot = sp.tile([128, 2, H, DR], BF16, tag="qpe_rot")
                qpe_f = sp.tile([128, 2, H * DR], BF16, tag="qpe_f")
                qT_stage = sp.tile([128, 24, SL], BF16, tag="qT_stage")
                # w_qb nope half resident (48K); rope third loaded after.
                # The nope part of q ships in its own AllToAll so the wire
                # overlaps the rope compute.
                for jq in range(2):
                    if jq == 1:
                        for ch in range(QC):
                            nc.scalar.dma_start(
                                out=wqb_sb[:, ch, :],
                                in_=wqb_v[:, ch, 1024:2048])
                    for r in range(2):
                        psb = [psA.tile([128, 512], F32, tag=f"a{2 * r + j}",
                                        name=f"psb{j}", bufs=1) for j in range(2)]
                        for ch in range(QC):
                            for j in range(2):
                                nc.tensor.matmul(
                                    psb[j][:], qaT[:, ch, r * 128:(r + 1) * 128],
                                    wqb_sb[:, ch, j * 512:(j + 1) * 512],
                                    start=(ch == 0), stop=(ch == QC - 1))
                        for j in range(2):
                            nc.vector.tensor_copy(
                                qn_sb[:, r, jq * 1024 + j * 512:
                                      jq * 1024 + (j + 1) * 512], psb[j][:])
                # nope transposes + ship + collective
                for r in range(2):
                    for ch in range(16):
                        pt = psA.tile([128, 128], BF16, tag="ptA", bufs=2)
                        nc.tensor.transpose(
                            pt[:], qn_sb[:, r, ch * 128:(ch + 1) * 128], ident_sb[:])
                        nc.vector.tensor_copy(
                            qT_stage[:, ch, r * 128:(r + 1) * 128], pt[:])
                for j in range(NCORES):
                    nc.sync.dma_start(
                        out=aqa_in[j * 2 * DN:(j + 1) * 2 * DN, :].rearrange(
                            "(c p) s -> p c s", p=128),
                        in_=qT_stage[:, 2 * j:2 * j + 2, :])
                cc("AllToAll", mybir.AluOpType.bypass, aqa_in, aqa_out)

                # rope third of w_qb
                wqb2_sb = sp.tile([128, QC, 2 * 512], BF16, tag="wqb2",
                                  name="wqb2_sb", bufs=1)
                for ch in range(QC):
                    nc.scalar.dma_start(
                        out=wqb2_sb[:, ch, :], in_=wqb_v[:, ch, 2048:3072])
                for r in range(2):
                    psb = [psA.tile([128, 512], F32, tag=f"a{4 + j}",
                                    name=f"psb2{j}", bufs=1) for j in range(2)]
                    for ch in range(QC):
                        for j in range(2):
                            nc.tensor.matmul(
                                psb[j][:], qaT[:, ch, r * 128:(r + 1) * 128],
                                wqb2_sb[:, ch, j * 512:(j + 1) * 512],
                                start=(ch == 0), stop=(ch == QC - 1))
                    for j in range(2):
                        nc.vector.tensor_copy(
                            qpe_raw[:, r, j * 512:(j + 1) * 512], psb[j][:])
                for r in range(2):
                    # rope on q_pe rows (all 16 heads at once, strided APs)
                    qpe_v = qpe_raw[:, r, :].rearrange("p (h d) -> p h d", h=H)
                    nc.vector.tensor_scalar_mul(
                        qpe_rot[:, r, :, 0:32], qpe_v[:, :, 32:64], -1.0)
                    nc.vector.tensor_copy(
                        qpe_rot[:, r, :, 32:64], qpe_v[:, :, 0:32])
                    t1q = sp.tile([128, H * DR], BF16, tag="t1q", bufs=1)
                    nc.vector.tensor_mul(t1q[:], qpe_raw[:, r, :], c16_sb[:, r, :])
                    t2q = sp.tile([128, H * DR], BF16, tag="t2q", bufs=1)
                    nc.vector.tensor_mul(
                        t2q[:],
                        qpe_rot[:, r, :, :].rearrange("p h d -> p (h d)"),
                        s16_sb[:, r, :])
                    nc.vector.tensor_add(qpe_f[:, r, :], t1q[:], t2q[:])
                    for ch in range(8):
                        pt = psA.tile([128, 128], BF16, tag="ptA", bufs=2)
                        nc.tensor.transpose(
                            pt[:], qpe_f[:, r, ch * 128:(ch + 1) * 128], ident_sb[:])
                        nc.vector.tensor_copy(
                            qT_stage[:, 16 + ch, r * 128:(r + 1) * 128], pt[:])
                for j in range(NCORES):
                    nc.sync.dma_start(
                        out=aqb_in[j * 2 * DR:(j + 1) * 2 * DR, :],
                        in_=qT_stage[:, 16 + j, :])
                cc("AllToAll", mybir.AluOpType.bypass, aqb_in, aqb_out)

            # ------------- attention phase (2 heads, full sequence) -------------
            with tc.tile_pool(name="att", bufs=1) as ap, \
                 tc.tile_pool(name="psB", bufs=1, space="PSUM") as psB:
                mask_sb = ap.tile([128, 4, 512], BF16, tag="mask")
                nc.sync.dma_start(
                    out=mask_sb, in_=masks.rearrange("p (m f) -> p m f", m=4))

                # gathered ckv/k_pe -> feature-major SBUF
                agv = ag_out.rearrange("(c r) s -> r c s", c=NCORES)  # [576,8,SL]
                ckvT = ap.tile([128, 4, S], BF16, tag="ckvT")
                for rc in range(4):
                    nc.scalar.dma_start(
                        out=ckvT[:, rc, :],
                        in_=agv[rc * 128:(rc + 1) * 128, :, :])
                kpeT = ap.tile([64, S], BF16, tag="kpeT")
                nc.scalar.dma_start(out=kpeT, in_=agv[R:AGR, :, :])

                # a2a-q outputs -> per-head q tiles
                aqav = aqa_out.rearrange("(c h p) s -> h p c s", c=NCORES, h=HPC)
                qnT = [ap.tile([128, S], BF16, tag=f"qnT{h}", name=f"qnT{h}")
                       for h in range(HPC)]
                for h in range(HPC):
                    nc.scalar.dma_start(out=qnT[h], in_=aqav[h])
                aqbv = aqb_out.rearrange("(c p) s -> p c s", p=128)
                qpeA = ap.tile([64, S], BF16, tag="qpeA")
                nc.scalar.dma_start(out=qpeA, in_=aqbv[0:64, :, :])
                qpeB = ap.tile([64, S], BF16, tag="qpeB")
                nc.scalar.dma_start(out=qpeB, in_=aqbv[64:128, :, :])
                # w_o flows during attention (the DMA-idle window); triggers
                # queue behind the q reassembly so it cannot crowd the sender.
                wo_sb = cp.tile([128, EC, E], BF16, tag="wo", name="wo_sb")
                wov = w_o.rearrange("(hc p) e2 -> p hc e2", p=128)
                for hc in range(EC):
                    nc.scalar.dma_start(out=wo_sb[:, hc, :], in_=wov[:, hc, :])

                # k_nope^T and v from gathered ckv
                wuk_sb = ap.tile([128, 4, HPC * DN], BF16, tag="wuk")
                nc.sync.dma_start(
                    out=wuk_sb, in_=w_uk.rearrange("p (rc m) -> p rc m", rc=4))
                wuv_sb = ap.tile([128, 4, HPC * DV], BF16, tag="wuv")
                nc.sync.dma_start(
                    out=wuv_sb, in_=w_uv.rearrange("p (rc m) -> p rc m", rc=4))
                knT = [ap.tile([128, S], BF16, tag=f"knT{h}", name=f"knT{h}")
                       for h in range(HPC)]
                for h in range(HPC):
                    psk = [psB.tile([128, 512], F32, tag=f"b{sc}", name=f"psk{sc}", bufs=1)
                           for sc in range(4)]
                    for rc in range(4):
                        for sc in range(4):
                            nc.tensor.matmul(
                                psk[sc][:], wuk_sb[:, rc, h * DN:(h + 1) * DN],
                                ckvT[:, rc, sc * 512:(sc + 1) * 512],
                                start=(rc == 0), stop=(rc == 3))
                    for sc in range(4):
                        nc.vector.tensor_copy(
                            knT[h][:, sc * 512:(sc + 1) * 512], psk[sc][:])
                v_sb = ap.tile([128, NKT, HPC * DV], BF16, tag="v_sb")
                for kt in range(NKT):
                    psv = psB.tile([128, HPC * DV], F32, tag="bE", bufs=2)
                    for rc in range(4):
                        nc.tensor.matmul(
                            psv[:], ckvT[:, rc, kt * 128:(kt + 1) * 128],
                            wuv_sb[:, rc, :], start=(rc == 0), stop=(rc == 3))
                    nc.vector.tensor_copy(v_sb[:, kt, :], psv[:])

                # --- causal attention, scoresT layout ---
                pending_drain = []

                def emit_drain():
                    while pending_drain:
                        fn = pending_drain.pop(0)
                        fn()

                for qc in range(NQC):
                    cs = slice(qc * 512, (qc + 1) * 512)
                    nkt = 4 * qc + 4
                    pos = [psB.tile([128, 512], F32, tag=f"b{h}", name=f"po{h}", bufs=1)
                           for h in range(HPC)]
                    pdns = [psB.tile([128, 512], F32, tag=f"b{2 + h}", name=f"pdn{h}", bufs=1)
                            for h in range(HPC)]
                    # software pipeline: scores(kt) issue before po/pdn(kt-1) so
                    # the PE never waits on the exp of the tile it just scored.
                    def emit_scores(kt):
                        ks = slice(kt * 128, (kt + 1) * 128)
                        pss = [psB.tile([128, 512], F32,
                                        tag=("bE" if h == 0 else "bF"),
                                        name=f"pss{h}", bufs=2)
                               for h in range(HPC)]
                        nc.tensor.matmul(pss[0][:], knT[0][:, ks], qnT[0][:, cs],
                                         start=True, stop=False)
                        nc.tensor.matmul(pss[1][:], knT[1][:, ks], qnT[1][:, cs],
                                         start=True, stop=False)
                        nc.tensor.matmul(pss[0][:], kpeT[:, ks], qpeA[:, cs],
                                         start=False, stop=True)
                        nc.tensor.matmul(pss[1][:], kpeT[:, ks], qpeB[:, cs],
                                         start=False, stop=True)
                        m = kt - 4 * qc
                        ets = []
                        for h in range(HPC):
                            if m >= 0:
                                nc.vector.tensor_add(pss[h][:], pss[h][:],
                                                     mask_sb[:, m, :])
                            et = ap.tile([128, 512], BF16, tag=f"et{h}",
                                         name=f"et{h}", bufs=4)
                            nc.scalar.activation(out=et, in_=pss[h][:], func=AF.Exp,
                                                 scale=SM_SCALE)
                            ets.append(et)
                        return ets

                    all_ets = {}

                    def emit_av(kt):
                        ets = all_ets[kt]
                        for h in range(HPC):
                            nc.tensor.matmul(
                                pos[h][:], v_sb[:, kt, h * DV:(h + 1) * DV],
                                ets[h][:], start=(kt == 0), stop=(kt == nkt - 1))
                        if kt % 2 == 1:
                            # sum the exp-tile pair on DVE, halve the ones-MMs
                            for h in range(HPC):
                                etp = ap.tile([128, 512], BF16, tag=f"etp{h}",
                                              name=f"etp{h}", bufs=2)
                                nc.vector.tensor_add(etp[:], ets[h][:],
                                                     all_ets[kt - 1][h][:])
                                nc.tensor.matmul(
                                    pdns[h][:], ones_sb[:], etp[:],
                                    start=(kt == 1), stop=(kt == nkt - 1))

                    all_ets[0] = emit_scores(0)
                    emit_drain()  # previous qc's divide runs under these scores
                    for kt in range(1, nkt):
                        all_ets[kt] = emit_scores(kt)
                        emit_av(kt - 1)
                    emit_av(nkt - 1)

                    def make_drain(qc, pos, pdns):
                        def drain():
                            lgs = []
                            for h in range(HPC):
                                lg = ap.tile([128, 512], F32, tag=f"lg{h}",
                                             name=f"lg{h}", bufs=2)
                                nc.scalar.activation(out=lg, in_=pdns[h][:],
                                                     func=AF.Ln)
                                lgs.append(lg)
                            for h in range(HPC):
                                rec = ap.tile([128, 512], F32, tag="rec", bufs=2)
                                nc.scalar.activation(out=rec, in_=lgs[h][:],
                                                     func=AF.Exp, scale=-1.0)
                                ofin = ap.tile([128, 512], BF16, tag=f"ofin{h}",
                                               name=f"ofin{h}", bufs=2)
                                nc.vector.tensor_mul(ofin[:], pos[h][:], rec[:])
                                for half in range(2):
                                    j = 2 * qc + half
                                    nc.sync.dma_start(
                                        out=ao_in[j * AOB + h * DV:
                                                  j * AOB + (h + 1) * DV, :],
                                        in_=ofin[:, half * 256:(half + 1) * 256])
                        return drain

                    pending_drain.append(make_drain(qc, pos, pdns))
                emit_drain()
                cc("AllToAll", mybir.AluOpType.bypass, ao_in, ao_out)

                # --- y slice: o_sl @ w_o (full), no reduction ---
                o_slT = ap.tile([128, EC, SL], BF16, tag="o_slT")
                nc.scalar.dma_start(
                    out=o_slT, in_=ao_out.rearrange("(hd p) s -> p hd s", p=128))
                # ~4us of junk matmuls to lift the HAM clock-gate back to full
                # rate before the y GEMM (PE sat idle through the o AllToAll).
                for w in range(20):
                    pjunk = psB.tile([128, SL], F32, tag="bE", name="pjunk", bufs=2)
                    nc.tensor.matmul(pjunk[:], ones_sb[:], o_slT[:, w % EC, :],
                                     start=True, stop=True)
                for qh in range(2):
                    pys = [psB.tile([128, 512], F32, tag=f"b{e2}", name=f"py{e2}", bufs=1)
                           for e2 in range(4)]
                    for hd in range(EC):
                        for e2 in range(4):
                            nc.tensor.matmul(
                                pys[e2][:], o_slT[:, hd, qh * 128:(qh + 1) * 128],
                                wo_sb[:, hd, e2 * 512:(e2 + 1) * 512],
                                start=(hd == 0), stop=(hd == EC - 1))
                    y_sb = ap.tile([128, E], F32, tag="y_sb", bufs=2)
                    for e2 in range(4):
                        nc.vector.tensor_copy(y_sb[:, e2 * 512:(e2 + 1) * 512],
                                              pys[e2][:])
                    nc.sync.dma_start(
                        out=y_sl[qh * 128:(qh + 1) * 128, :], in_=y_sb[:])
    nc.finalize()
    return nc


_NC_CACHE = None


def _get_nc():
    global _NC_CACHE
    if _NC_CACHE is None:
        _NC_CACHE = _build()
    return _NC_CACHE


def _make_in_maps(x, w_q_a, q_a_ln_w, w_q_b, w_kv_a, kv_a_ln_w, w_kv_b, w_o):
    x = np.asarray(x, dtype=np.float32)
    w_q_a = np.asarray(w_q_a, dtype=np.float32)
    q_a_ln_w = np.asarray(q_a_ln_w, dtype=np.float32)
    w_q_b = np.asarray(w_q_b, dtype=np.float32)
    w_kv_a = np.asarray(w_kv_a, dtype=np.float32)
    kv_a_ln_w = np.asarray(kv_a_ln_w, dtype=np.float32)
    w_kv_b = np.asarray(w_kv_b, dtype=np.float32)
    w_o = np.asarray(w_o, dtype=np.float32)

    bf = lambda a: np.ascontiguousarray(a.astype(BFNP))

    # fold q_a layernorm weight into w_q_b rows; reorder cols [nope|rope]
    wqb_eff = w_q_b * q_a_ln_w[:, None]
    wqb3 = wqb_eff.reshape(QLR, H, DN + DR)
    wqb_r = np.concatenate(
        [wqb3[:, :, :DN].reshape(QLR, H * DN),
         wqb3[:, :, DN:].reshape(QLR, H * DR)], axis=1)

    # fold kv layernorm weight into w_uk / w_uv
    wkv3 = w_kv_b.reshape(R, H, DN + DV) * kv_a_ln_w[:, None, None]
    cos_rm, sin_rm = _rope_rm()
    cos16 = np.tile(cos_rm, (1, H))
    sin16 = np.tile(sin_rm, (1, H))
    masks = _masks().reshape(4 * 128, 512)

    wqa_b = bf(w_q_a)
    wkva_b = bf(w_kv_a)
    wqb_b = bf(wqb_r)
    wo_b = bf(w_o)
    ones_b = np.ones((128, 128), dtype=BFNP)
    ident_b = np.eye(128, dtype=BFNP)

    def pm(a, p=128):
        # [n*128, m] row-chunked -> partition-major [128, n*m]
        n = a.shape[0] // p
        return np.ascontiguousarray(
            a.reshape(n, p, a.shape[1]).transpose(1, 0, 2).reshape(p, -1))

    masks_pm = pm(masks)
    in_maps = []
    for c in range(NCORES):
        h0 = HPC * c
        sl = slice(c * SL, (c + 1) * SL)
        in_maps.append({
            "xT_pm": pm(bf(x[0, sl, :].T)),
            "w_qa": wqa_b,
            "w_kva_pm": pm(wkva_b),
            "w_qb": wqb_b,
            "w_uk_pm": pm(bf(np.concatenate(
                [wkv3[:, h0, :DN], wkv3[:, h0 + 1, :DN]], axis=1))),
            "w_uv_pm": pm(bf(np.concatenate(
                [wkv3[:, h0, DN:], wkv3[:, h0 + 1, DN:]], axis=1))),
            "w_o_f": wo_b,
            "cosq_pm": pm(bf(cos_rm[sl])),
            "sinq_pm": pm(bf(sin_rm[sl])),
            "cos16_pm": pm(bf(cos16[sl])),
            "sin16_pm": pm(bf(sin16[sl])),
            "masks_pm": masks_pm,
            "ones_in": ones_b,
            "ident_in": ident_b,
        })
    return in_maps


def kernel(**inputs):
    in_maps = _make_in_maps(**inputs)
    nc = _get_nc()
    # The axon terminal occasionally reports NRT_EXEC_UNIT_UNRECOVERABLE on the
    # first load after a prior session died; a retry recovers it.
    last_exc = None
    for _ in range(3):
        try:
            res = run_bass_kernel_spmd(nc, in_maps, core_ids=list(range(NCORES)))
            break
        except Exception as e:  # noqa: BLE001
            last_exc = e
    else:
        raise last_exc
    y = np.concatenate([res.results[c]["y_sl"] for c in range(NCORES)], axis=0)
    return y.reshape(B, S, E).astype(np.float32)


if __name__ == "__main__":
    nc = _build()
    print("built ok")


# revision 28
# speedup vs baseline: 1.0278x; 1.0278x over previous
"""DeepseekV2 MLA attention (prefill, causal) on 8 trn2 NeuronCores — v2.

Math: non-absorbed form (optimal for prefill):
    k_nope = ckv @ w_uk,  v = ckv @ w_uv          (per head)
    scores = [q_nope;q_pe] . [k_nope;k_pe]        (d = 192)
    out    = softmax(scores) @ v ;  y = concat_h(out) @ w_o

Sharding & wire plan (replaces v1's AllGather(q_a)+ReduceScatter(y)):
  - Projections are sequence-sharded (256 rows/core); attention is
    head-sharded (2 heads/core); y is sequence-sharded again.
  - AllGather moves only ckv+k_pe (576x256 bf16 = 0.3 MB/core).
  - q is projected for ALL 16 heads on the row-owning core, then
    AllToAll'd to the head-owning cores in two waves (nope 1 MB, rope
    0.5 MB bf16, mesh one-hop) so the wire overlaps the rope compute.
  - o is AllToAll'd back to row owners (1 MB bf16); each core computes
    its own 256-row slice of y with the full (bf16) w_o.  No reduce.
  - RMSNorm weights are folded into the downstream matmuls on the host;
    x is pre-transposed and laid out partition-major on the host (big
    contiguous DMA lines).  All PE stationary operands are bf16 so the
    compiler's fast-weight-load kicks in; PSUM stays f32.
  - DMA triggers are spread over the sync/scalar/gpsimd queues and
    ordered so bulk weights (w_qb, w_o) cannot starve the critical
    path; w_o streams during attention's DMA-idle window.  The softmax
    divide of chunk qc issues under chunk qc+1's first score matmuls;
    1/sum comes from exp(-ln(sum)) on the scalar engine.
"""
import sys

sys.path.insert(0, "/opt/trn_rl_repo")

import numpy as np
import ml_dtypes

import concourse.bass as bass
from concourse import bacc
import concourse.mybir as mybir
import concourse.tile as tile
from concourse.bass_utils import run_bass_kernel_spmd

F32 = mybir.dt.float32
BF16 = mybir.dt.bfloat16
AF = mybir.ActivationFunctionType
BFNP = ml_dtypes.bfloat16

B, S, E, H = 1, 2048, 2048, 16
DN, DR, DV, R, QLR = 128, 64, 128, 512, 1536
EPS = 1e-6
NCORES = 8
SL = S // NCORES          # 256 sequence rows per core
HPC = H // NCORES         # 2 heads per core
SM_SCALE = (DN + DR) ** -0.5
NEG = -1e30
ROPE_BASE = 10000.0

EC = E // 128             # 16 contraction chunks over E
QC = QLR // 128           # 12 chunks over QLR
NQC = S // 512            # 4 query column chunks
NKT = S // 128            # 16 key tiles
AGR = R + DR              # 576 rows in the allgather payload
AQB = 2 * DN + 2 * DR     # 384 rows per a2a-q shard (2 heads nope + pe)
AQR = NCORES * AQB        # 3072
AOB = HPC * DV            # 256 rows per a2a-o shard
AOR = NCORES * AOB        # 2048


def _rope_rm():
    """Row-major cos/sin tables [S, DR] (fp64 -> f32)."""
    inv_freq = 1.0 / (ROPE_BASE ** (np.arange(0, DR, 2, dtype=np.float64) / DR))
    ang = np.arange(S, dtype=np.float64)[:, None] * inv_freq[None, :]
    cos = np.concatenate([np.cos(ang), np.cos(ang)], -1).astype(np.float32)
    sin = np.concatenate([np.sin(ang), np.sin(ang)], -1).astype(np.float32)
    return cos, sin  # [S, 64]


def _masks():
    # scoresT tile [k 128 | q 512]; m = kt - 4*qc; valid iff q >= k
    ii = np.arange(128)[:, None]
    jj = np.arange(512)[None, :]
    return np.stack(
        [np.where(jj - ii - 128 * m >= 0, 0.0, NEG).astype(BFNP) for m in range(4)]
    )  # [4,128,512]


def _build(skip_collectives=False):
    nc = bacc.Bacc(None, num_devices=NCORES)

    # partition-major [128, ...] layouts (host-prepared) so each load is one
    # DMA with multi-KB contiguous lines per partition, not 512B descriptors
    xT = nc.dram_tensor("xT_pm", [128, EC * SL], BF16, kind="ExternalInput")
    w_qa = nc.dram_tensor("w_qa", [E, QLR], BF16, kind="ExternalInput")
    w_kva = nc.dram_tensor("w_kva_pm", [128, EC * (R + DR)], BF16,
                           kind="ExternalInput")
    w_qb = nc.dram_tensor("w_qb", [QLR, H * (DN + DR)], BF16, kind="ExternalInput")
    w_uk = nc.dram_tensor("w_uk_pm", [128, 4 * HPC * DN], BF16,
                          kind="ExternalInput")
    w_uv = nc.dram_tensor("w_uv_pm", [128, 4 * HPC * DV], BF16,
                          kind="ExternalInput")
    w_o = nc.dram_tensor("w_o_f", [H * DV, E], BF16, kind="ExternalInput")
    cosq = nc.dram_tensor("cosq_pm", [128, 2 * DR], BF16, kind="ExternalInput")
    sinq = nc.dram_tensor("sinq_pm", [128, 2 * DR], BF16, kind="ExternalInput")
    cos16 = nc.dram_tensor("cos16_pm", [128, 2 * H * DR], BF16,
                           kind="ExternalInput")
    sin16 = nc.dram_tensor("sin16_pm", [128, 2 * H * DR], BF16,
                           kind="ExternalInput")
    masks = nc.dram_tensor("masks_pm", [128, 4 * 512], BF16, kind="ExternalInput")
    ones_in = nc.dram_tensor("ones_in", [128, 128], BF16, kind="ExternalInput")
    ident_in = nc.dram_tensor("ident_in", [128, 128], BF16, kind="ExternalInput")
    y_sl = nc.dram_tensor("y_sl", [SL, E], F32, kind="ExternalOutput")

    ag_in = nc.dram_tensor("ag_in", [AGR, SL], BF16)
    ag_out = nc.dram_tensor("ag_out", [NCORES * AGR, SL], BF16, addr_space="Shared")
    aqa_in = nc.dram_tensor("aqa_in", [NCORES * 2 * DN, SL], BF16)
    aqa_out = nc.dram_tensor("aqa_out", [NCORES * 2 * DN, SL], BF16)
    aqb_in = nc.dram_tensor("aqb_in", [NCORES * 2 * DR, SL], BF16)
    aqb_out = nc.dram_tensor("aqb_out", [NCORES * 2 * DR, SL], BF16)
    ao_in = nc.dram_tensor("ao_in", [AOR, SL], BF16)
    ao_out = nc.dram_tensor("ao_out", [AOR, SL], BF16)

    def cc(kind, op, in_t, out_t):
        if skip_collectives:
            if kind == "AllGather":
                for c in range(NCORES):
                    rows = in_t.shape[0]
                    nc.gpsimd.dma_start(
                        out=out_t[c * rows:(c + 1) * rows, :], in_=in_t[:, :])
            else:
                nc.gpsimd.dma_start(out=out_t[:, :], in_=in_t[:, :])
        else:
            nc.gpsimd.collective_compute(
                kind, op, replica_groups=[list(range(NCORES))],
                ins=[in_t[:, :].opt()], outs=[out_t[:, :].opt()])

    with tile.TileContext(nc) as tc:
        with tc.tile_pool(name="consts", bufs=1) as cp:
            ones_sb = cp.tile([128, 128], BF16)
            nc.sync.dma_start(out=ones_sb, in_=ones_in[:, :])
            ident_sb = cp.tile([128, 128], BF16)
            nc.sync.dma_start(out=ident_sb, in_=ident_in[:, :])
            eps_sb = cp.tile([128, 1], F32)
            nc.vector.memset(eps_sb[:], EPS)

            # ------------- sender phase: projections on my 256 rows -------------
            with tc.tile_pool(name="snd", bufs=1) as sp, \
                 tc.tile_pool(name="psA", bufs=1, space="PSUM") as psA:
                xT_sb = sp.tile([128, EC, SL], BF16, tag="xT")
                xT_v = xT.rearrange("p (ec s) -> p ec s", ec=EC)
                nc.sync.dma_start(out=xT_sb[:, 0:8, :], in_=xT_v[:, 0:8, :])
                nc.scalar.dma_start(out=xT_sb[:, 8:EC, :], in_=xT_v[:, 8:EC, :])

                # --- ckv + k_pe (row-major), rmsnorm, rope; ship to AllGather ---
                ps_ckv = [psA.tile([128, 512], F32, tag=f"a{r}", name=f"psckv{r}", bufs=1)
                          for r in range(2)]
                ps_pe = [psA.tile([128, DR], F32, tag=f"a{2 + r}", name=f"pspe{r}", bufs=1)
                         for r in range(2)]
                wkva_v = w_kva.rearrange("p (ec m) -> p ec m", ec=EC)
                for e in range(EC):
                    wk = sp.tile([128, R + DR], BF16, tag="wkva", bufs=6)
                    nc.sync.dma_start(out=wk, in_=wkva_v[:, e, :])
                    for r in range(2):
                        rs = slice(r * 128, (r + 1) * 128)
                        nc.tensor.matmul(ps_ckv[r][:], xT_sb[:, e, rs], wk[:, 0:R],
                                         start=(e == 0), stop=(e == EC - 1))
                        nc.tensor.matmul(ps_pe[r][:], xT_sb[:, e, rs], wk[:, R:],
                                         start=(e == 0), stop=(e == EC - 1))
                cosq_sb = sp.tile([128, 2, DR], BF16, tag="cosq")
                nc.sync.dma_start(
                    out=cosq_sb, in_=cosq.rearrange("p (r d) -> p r d", r=2))
                sinq_sb = sp.tile([128, 2, DR], BF16, tag="sinq")
                nc.sync.dma_start(
                    out=sinq_sb, in_=sinq.rearrange("p (r d) -> p r d", r=2))
                ckv_n = sp.tile([128, 2, R], BF16, tag="ckv_n")
                kpe_o = sp.tile([128, 2, DR], BF16, tag="kpe_o")
                sq_junk = sp.tile([128, 512], F32, tag="sq_junk", bufs=1)
                for r in range(2):
                    ssq = sp.tile([128, 1], F32, tag="ssq", bufs=2)
                    nc.scalar.activation(out=sq_junk, in_=ps_ckv[r][:],
                                         func=AF.Square, accum_out=ssq[:])
                    rstd = sp.tile([128, 1], F32, tag="rstd", bufs=2)
                    nc.scalar.activation(out=rstd, in_=ssq[:], func=AF.Sqrt,
                                         scale=1.0 / R, bias=eps_sb[:])
                    nc.vector.reciprocal(rstd[:], rstd[:])
                    nc.vector.tensor_scalar_mul(ckv_n[:, r, :], ps_ckv[r][:], rstd[:])
                    # rope on k_pe (row-major: shuffle along free dim)
                    rot = sp.tile([128, DR], F32, tag="rot", bufs=2)
                    nc.vector.tensor_scalar_mul(rot[:, 0:32], ps_pe[r][:, 32:64], -1.0)
                    nc.vector.tensor_copy(rot[:, 32:64], ps_pe[r][:, 0:32])
                    t1 = sp.tile([128, DR], F32, tag="t1", bufs=2)
                    nc.vector.tensor_mul(t1[:], ps_pe[r][:], cosq_sb[:, r, :])
                    t2 = sp.tile([128, DR], F32, tag="t2", bufs=2)
                    nc.vector.tensor_mul(t2[:], rot[:], sinq_sb[:, r, :])
                    nc.vector.tensor_add(kpe_o[:, r, :], t1[:], t2[:])
                # transpose [2*128, 576] -> [576, 256] and ship
                ag_stage = sp.tile([128, 4, SL], BF16, tag="ag_stage")
                agpe_stage = sp.tile([64, SL], BF16, tag="agpe_stage")
                for r in range(2):
                    for cch in range(4):
                        pt = psA.tile([128, 128], BF16, tag="ptA", bufs=2)
                        nc.tensor.transpose(
                            pt[:], ckv_n[:, r, cch * 128:(cch + 1) * 128], ident_sb[:])
                        nc.vector.tensor_copy(
                            ag_stage[:, cch, r * 128:(r + 1) * 128], pt[:])
                    ptp = psA.tile([64, 128], BF16, tag="ptA", bufs=2)
                    nc.tensor.transpose(ptp[:], kpe_o[:, r, :], ident_sb[:])
                    nc.vector.tensor_copy(agpe_stage[:, r * 128:(r + 1) * 128], ptp[:])
                nc.gpsimd.dma_start(
                    out=ag_in[0:R, :].rearrange("(rc p) s -> p rc s", p=128),
                    in_=ag_stage[:])
                nc.gpsimd.dma_start(out=ag_in[R:AGR, :], in_=agpe_stage[:])
                cc("AllGather", mybir.AluOpType.bypass, ag_in, ag_out)

                wqb_v = w_qb.rearrange("(qc p) m -> p qc m", p=128)

                # w_qb nope block loads early on the scalar queue
                # (fires after the ckv rmsnorm ACTs; fully resident).
                wqb_sb = sp.tile([128, QC, 4 * 512], BF16, tag="wqb",
                                 name="wqb_sb", bufs=1)
                for ch in range(QC):
                    nc.scalar.dma_start(
                        out=wqb_sb[:, ch, :], in_=wqb_v[:, ch, 0:2048])

                # --- q_a (row-major) + rmsnorm ---
                psq = [psA.tile([128, 512], F32, tag=f"a{i}", name=f"psq{i}", bufs=1)
                       for i in range(6)]
                wqa_v = w_qa.rearrange("(ec p) m -> p ec m", p=128)
                for e in range(EC):
                    wq = sp.tile([128, QLR], BF16, tag="wqa", bufs=6)
                    nc.gpsimd.dma_start(out=wq, in_=wqa_v[:, e, :])
                    for r in range(2):
                        rs = slice(r * 128, (r + 1) * 128)
                        for j in range(3):
                            nc.tensor.matmul(
                                psq[3 * r + j][:], xT_sb[:, e, rs],
                                wq[:, j * 512:(j + 1) * 512],
                                start=(e == 0), stop=(e == EC - 1))
                qa_n = sp.tile([128, 2, QLR], BF16, tag="qa_n")
                for r in range(2):
                    ssqs = []
                    for j in range(3):
                        ssq = sp.tile([128, 1], F32, tag=f"ssqq{j}", bufs=2)
                        nc.scalar.activation(out=sq_junk, in_=psq[3 * r + j][:],
                                             func=AF.Square, accum_out=ssq[:])
                        ssqs.append(ssq)
                    nc.vector.tensor_add(ssqs[0][:], ssqs[0][:], ssqs[1][:])
                    nc.vector.tensor_add(ssqs[0][:], ssqs[0][:], ssqs[2][:])
                    rstd = sp.tile([128, 1], F32, tag="rstdq", bufs=2)
                    nc.scalar.activation(out=rstd, in_=ssqs[0][:], func=AF.Sqrt,
                                         scale=1.0 / QLR, bias=eps_sb[:])
                    nc.vector.reciprocal(rstd[:], rstd[:])
                    for j in range(3):
                        nc.vector.tensor_scalar_mul(
                            qa_n[:, r, j * 512:(j + 1) * 512], psq[3 * r + j][:],
                            rstd[:])
                # transpose q_a -> q_aT [1536, 256] (stationary for q_b)
                qaT = sp.tile([128, QC, SL], BF16, tag="qaT")
                for r in range(2):
                    for ch in range(QC):
                        pt = psA.tile([128, 128], BF16, tag="ptA", bufs=2)
                        nc.tensor.transpose(
                            pt[:], qa_n[:, r, ch * 128:(ch + 1) * 128], ident_sb[:])
                        nc.vector.tensor_copy(qaT[:, ch, r * 128:(r + 1) * 128], pt[:])

                # --- q = q_aT.T @ w_qb (row-major out), rope, transpose, ship ---
                qn_sb = sp.tile([128, 2, H * DN], BF16, tag="qn_sb")
                qpe_raw = sp.tile([128, 2, H * DR], BF16, tag="qpe_raw")
                c16_sb = sp.tile([128, 2, H * DR], BF16, tag="c16")
                nc.sync.dma_start(
                    out=c16_sb, in_=cos16.rearrange("p (r d) -> p r d", r=2))
                s16_sb = sp.tile([128, 2, H * DR], BF16, tag="s16")
                nc.sync.dma_start(
                    out=s16_sb, in_=sin16.rearrange("p (r d) -> p r d", r=2))
                qpe_rot = sp.tile([128, 2, H, DR], BF16, tag="qpe_rot")
                qpe_f = sp.tile([128, 2, H * DR], BF16, tag="qpe_f")
                qT_stage = sp.tile([128, 24, SL], BF16, tag="qT_stage")
                # w_qb nope half resident (48K); rope third loaded after.
                # The nope part of q ships in its own AllToAll so the wire
                # overlaps the rope compute.
                for r in range(2):
                    psb = [psA.tile([128, 512], F32, tag=f"a{j}",
                                    name=f"psb{j}", bufs=1) for j in range(4)]
                    for ch in range(QC):
                        for j in range(4):
                            nc.tensor.matmul(
                                psb[j][:], qaT[:, ch, r * 128:(r + 1) * 128],
                                wqb_sb[:, ch, j * 512:(j + 1) * 512],
                                start=(ch == 0), stop=(ch == QC - 1))
                    for j in range(4):
                        nc.vector.tensor_copy(
                            qn_sb[:, r, j * 512:(j + 1) * 512], psb[j][:])
                # nope transposes + ship + collective
                for r in range(2):
                    for ch in range(16):
                        pt = psA.tile([128, 128], BF16, tag="ptA", bufs=2)
                        nc.tensor.transpose(
                            pt[:], qn_sb[:, r, ch * 128:(ch + 1) * 128], ident_sb[:])
                        nc.vector.tensor_copy(
                            qT_stage[:, ch, r * 128:(r + 1) * 128], pt[:])
                for j in range(NCORES):
                    nc.sync.dma_start(
                        out=aqa_in[j * 2 * DN:(j + 1) * 2 * DN, :].rearrange(
                            "(c p) s -> p c s", p=128),
                        in_=qT_stage[:, 2 * j:2 * j + 2, :])
                cc("AllToAll", mybir.AluOpType.bypass, aqa_in, aqa_out)

                # rope third of w_qb
                wqb2_sb = sp.tile([128, QC, 2 * 512], BF16, tag="wqb2",
                                  name="wqb2_sb", bufs=1)
                for ch in range(QC):
                    nc.scalar.dma_start(
                        out=wqb2_sb[:, ch, :], in_=wqb_v[:, ch, 2048:3072])
                for r in range(2):
                    psb = [psA.tile([128, 512], F32, tag=f"a{4 + j}",
                                    name=f"psb2{j}", bufs=1) for j in range(2)]
                    for ch in range(QC):
                        for j in range(2):
                            nc.tensor.matmul(
                                psb[j][:], qaT[:, ch, r * 128:(r + 1) * 128],
                                wqb2_sb[:, ch, j * 512:(j + 1) * 512],
                                start=(ch == 0), stop=(ch == QC - 1))
                    for j in range(2):
                        nc.vector.tensor_copy(
                            qpe_raw[:, r, j * 512:(j + 1) * 512], psb[j][:])
                for r in range(2):
                    # rope on q_pe rows (all 16 heads at once, strided APs)
                    qpe_v = qpe_raw[:, r, :].rearrange("p (h d) -> p h d", h=H)
                    nc.vector.tensor_scalar_mul(
                        qpe_rot[:, r, :, 0:32], qpe_v[:, :, 32:64], -1.0)
                    nc.vector.tensor_copy(
                        qpe_rot[:, r, :, 32:64], qpe_v[:, :, 0:32])
                    t1q = sp.tile([128, H * DR], BF16, tag="t1q", bufs=1)
                    nc.vector.tensor_mul(t1q[:], qpe_raw[:, r, :], c16_sb[:, r, :])
                    t2q = sp.tile([128, H * DR], BF16, tag="t2q", bufs=1)
                    nc.vector.tensor_mul(
                        t2q[:],
                        qpe_rot[:, r, :, :].rearrange("p h d -> p (h d)"),
                        s16_sb[:, r, :])
                    nc.vector.tensor_add(qpe_f[:, r, :], t1q[:], t2q[:])
                    for ch in range(8):
                        pt = psA.tile([128, 128], BF16, tag="ptA", bufs=2)
                        nc.tensor.transpose(
                            pt[:], qpe_f[:, r, ch * 128:(ch + 1) * 128], ident_sb[:])
                        nc.vector.tensor_copy(
                            qT_stage[:, 16 + ch, r * 128:(r + 1) * 128], pt[:])
                for j in range(NCORES):
                    nc.sync.dma_start(
                        out=aqb_in[j * 2 * DR:(j + 1) * 2 * DR, :],
                        in_=qT_stage[:, 16 + j, :])
                cc("AllToAll", mybir.AluOpType.bypass, aqb_in, aqb_out)

            # ------------- attention phase (2 heads, full sequence) -------------
            with tc.tile_pool(name="att", bufs=1) as ap, \
                 tc.tile_pool(name="psB", bufs=1, space="PSUM") as psB:
                mask_sb = ap.tile([128, 4, 512], BF16, tag="mask")
                nc.sync.dma_start(
                    out=mask_sb, in_=masks.rearrange("p (m f) -> p m f", m=4))

                # gathered ckv/k_pe -> feature-major SBUF
                agv = ag_out.rearrange("(c r) s -> r c s", c=NCORES)  # [576,8,SL]
                ckvT = ap.tile([128, 4, S], BF16, tag="ckvT")
                for rc in range(4):
                    nc.scalar.dma_start(
                        out=ckvT[:, rc, :],
                        in_=agv[rc * 128:(rc + 1) * 128, :, :])
                kpeT = ap.tile([64, S], BF16, tag="kpeT")
                nc.scalar.dma_start(out=kpeT, in_=agv[R:AGR, :, :])

                # a2a-q outputs -> per-head q tiles
                aqav = aqa_out.rearrange("(c h p) s -> h p c s", c=NCORES, h=HPC)
                qnT = [ap.tile([128, S], BF16, tag=f"qnT{h}", name=f"qnT{h}")
                       for h in range(HPC)]
                for h in range(HPC):
                    nc.scalar.dma_start(out=qnT[h], in_=aqav[h])
                aqbv = aqb_out.rearrange("(c p) s -> p c s", p=128)
                qpeA = ap.tile([64, S], BF16, tag="qpeA")
                nc.scalar.dma_start(out=qpeA, in_=aqbv[0:64, :, :])
                qpeB = ap.tile([64, S], BF16, tag="qpeB")
                nc.scalar.dma_start(out=qpeB, in_=aqbv[64:128, :, :])
                # w_o flows during attention (the DMA-idle window); triggers
                # queue behind the q reassembly so it cannot crowd the sender.
                wo_sb = ap.tile([128, EC, E], BF16, tag="wo", name="wo_sb")
                wov = w_o.rearrange("(hc p) e2 -> p hc e2", p=128)
                for hc in range(EC):
                    nc.scalar.dma_start(out=wo_sb[:, hc, :], in_=wov[:, hc, :])

                # k_nope^T and v from gathered ckv
                wuk_sb = ap.tile([128, 4, HPC * DN], BF16, tag="wuk")
                nc.sync.dma_start(
                    out=wuk_sb, in_=w_uk.rearrange("p (rc m) -> p rc m", rc=4))
                wuv_sb = ap.tile([128, 4, HPC * DV], BF16, tag="wuv")
                nc.sync.dma_start(
                    out=wuv_sb, in_=w_uv.rearrange("p (rc m) -> p rc m", rc=4))
                knT = [ap.tile([128, S], BF16, tag=f"knT{h}", name=f"knT{h}")
                       for h in range(HPC)]
                for h in range(HPC):
                    psk = [psB.tile([128, 512], F32, tag=f"b{sc}", name=f"psk{sc}", bufs=1)
                           for sc in range(4)]
                    for rc in range(4):
                        for sc in range(4):
                            nc.tensor.matmul(
                                psk[sc][:], wuk_sb[:, rc, h * DN:(h + 1) * DN],
                                ckvT[:, rc, sc * 512:(sc + 1) * 512],
                                start=(rc == 0), stop=(rc == 3))
                    for sc in range(4):
                        nc.vector.tensor_copy(
                            knT[h][:, sc * 512:(sc + 1) * 512], psk[sc][:])
                v_sb = ap.tile([128, NKT, HPC * DV], BF16, tag="v_sb")
                for kt in range(NKT):
                    psv = psB.tile([128, HPC * DV], F32, tag="bE", bufs=2)
                    for rc in range(4):
                        nc.tensor.matmul(
                            psv[:], ckvT[:, rc, kt * 128:(kt + 1) * 128],
                            wuv_sb[:, rc, :], start=(rc == 0), stop=(rc == 3))
                    nc.vector.tensor_copy(v_sb[:, kt, :], psv[:])

                # --- causal attention, scoresT layout ---
                pending_drain = []

                def emit_drain():
                    while pending_drain:
                        fn = pending_drain.pop(0)
                        fn()

                for qc in range(NQC):
                    cs = slice(qc * 512, (qc + 1) * 512)
                    nkt = 4 * qc + 4
                    pos = [psB.tile([128, 512], F32, tag=f"b{h}", name=f"po{h}", bufs=1)
                           for h in range(HPC)]
                    pdns = [psB.tile([128, 512], F32, tag=f"b{2 + h}", name=f"pdn{h}", bufs=1)
                            for h in range(HPC)]
                    # software pipeline: scores(kt) issue before po/pdn(kt-1) so
                    # the PE never waits on the exp of the tile it just scored.
                    def emit_scores(kt):
                        ks = slice(kt * 128, (kt + 1) * 128)
                        pss = [psB.tile([128, 512], F32,
                                        tag=("bE" if h == 0 else "bF"),
                                        name=f"pss{h}", bufs=2)
                               for h in range(HPC)]
                        nc.tensor.matmul(pss[0][:], knT[0][:, ks], qnT[0][:, cs],
                                         start=True, stop=False)
                        nc.tensor.matmul(pss[1][:], knT[1][:, ks], qnT[1][:, cs],
                                         start=True, stop=False)
                        nc.tensor.matmul(pss[0][:], kpeT[:, ks], qpeA[:, cs],
                                         start=False, stop=True)
                        nc.tensor.matmul(pss[1][:], kpeT[:, ks], qpeB[:, cs],
                                         start=False, stop=True)
                        m = kt - 4 * qc
                        ets = []
                        for h in range(HPC):
                            if m >= 0:
                                nc.vector.tensor_add(pss[h][:], pss[h][:],
                                                     mask_sb[:, m, :])
                            et = ap.tile([128, 512], BF16, tag=f"et{h}",
                                         name=f"et{h}", bufs=4)
                            nc.scalar.activation(out=et, in_=pss[h][:], func=AF.Exp,
                                                 scale=SM_SCALE)
                            ets.append(et)
                        return ets

                    all_ets = {}

                    def emit_av(kt):
                        ets = all_ets[kt]
                        for h in range(HPC):
                            nc.tensor.matmul(
                                pos[h][:], v_sb[:, kt, h * DV:(h + 1) * DV],
                                ets[h][:], start=(kt == 0), stop=(kt == nkt - 1))
                        if kt % 2 == 1:
                            # sum the exp-tile pair on DVE, halve the ones-MMs
                            for h in range(HPC):
                                etp = ap.tile([128, 512], BF16, tag=f"etp{h}",
                                              name=f"etp{h}", bufs=2)
                                nc.vector.tensor_add(etp[:], ets[h][:],
                                                     all_ets[kt - 1][h][:])
                                nc.tensor.matmul(
                                    pdns[h][:], ones_sb[:], etp[:],
                                    start=(kt == 1), stop=(kt == nkt - 1))

                    all_ets[0] = emit_scores(0)
                    emit_drain()  # previous qc's divide runs under these scores
                    for kt in range(1, nkt):
                        all_ets[kt] = emit_scores(kt)
                        emit_av(kt - 1)
                    emit_av(nkt - 1)

                    def make_drain(qc, pos, pdns):
                        def drain():
                            lgs = []
                            for h in range(HPC):
                                lg = ap.tile([128, 512], F32, tag=f"lg{h}",
                                             name=f"lg{h}", bufs=2)
                                nc.scalar.activation(out=lg, in_=pdns[h][:],
                                                     func=AF.Ln)
                                lgs.append(lg)
                            for h in range(HPC):
                                rec = ap.tile([128, 512], F32, tag="rec", bufs=2)
                                nc.scalar.activation(out=rec, in_=lgs[h][:],
                                                     func=AF.Exp, scale=-1.0)
                                ofin = ap.tile([128, 512], BF16, tag=f"ofin{h}",
                                               name=f"ofin{h}", bufs=2)
                                nc.vector.tensor_mul(ofin[:], pos[h][:], rec[:])
                                for half in range(2):
                                    j = 2 * qc + half
                                    nc.sync.dma_start(
                                        out=ao_in[j * AOB + h * DV:
                                                  j * AOB + (h + 1) * DV, :],
                                        in_=ofin[:, half * 256:(half + 1) * 256])
                        return drain

                    pending_drain.append(make_drain(qc, pos, pdns))
                emit_drain()
                cc("AllToAll", mybir.AluOpType.bypass, ao_in, ao_out)

                # --- y slice: o_sl @ w_o (full), no reduction ---
                o_slT = ap.tile([128, EC, SL], BF16, tag="o_slT")
                ao_v = ao_out.rearrange("(hd p) s -> p hd s", p=128)
                nc.sync.dma_start(out=o_slT[:, 0:8, :], in_=ao_v[:, 0:8, :])
                nc.scalar.dma_start(out=o_slT[:, 8:EC, :], in_=ao_v[:, 8:EC, :])
                # ~4us of junk matmuls to lift the HAM clock-gate back to full
                # rate before the y GEMM (PE sat idle through the o AllToAll).
                for w in range(20):
                    pjunk = psB.tile([128, SL], F32, tag="bE", name="pjunk", bufs=2)
                    nc.tensor.matmul(pjunk[:], ones_sb[:], o_slT[:, w % EC, :],
                                     start=True, stop=True)
                for qh in range(2):
                    pys = [psB.tile([128, 512], F32, tag=f"b{e2}", name=f"py{e2}", bufs=1)
                           for e2 in range(4)]
                    for hd in range(EC):
                        for e2 in range(4):
                            nc.tensor.matmul(
                                pys[e2][:], o_slT[:, hd, qh * 128:(qh + 1) * 128],
                                wo_sb[:, hd, e2 * 512:(e2 + 1) * 512],
                                start=(hd == 0), stop=(hd == EC - 1))
                    y_sb = ap.tile([128, E], F32, tag="y_sb", bufs=2)
                    for e2 in range(4):
                        nc.vector.tensor_copy(y_sb[:, e2 * 512:(e2 + 1) * 512],
                                              pys[e2][:])
                    nc.sync.dma_start(
                        out=y_sl[qh * 128:(qh + 1) * 128, :], in_=y_sb[:])
    nc.finalize()
    return nc


_NC_CACHE = None


def _get_nc():
    global _NC_CACHE
    if _NC_CACHE is None:
        _NC_CACHE = _build()
    return _NC_CACHE


def _make_in_maps(x, w_q_a, q_a_ln_w, w_q_b, w_kv_a, kv_a_ln_w, w_kv_b, w_o):
    x = np.asarray(x, dtype=np.float32)
    w_q_a = np.asarray(w_q_a, dtype=np.float32)
    q_a_ln_w = np.asarray(q_a_ln_w, dtype=np.float32)
    w_q_b = np.asarray(w_q_b, dtype=np.float32)
    w_kv_a = np.asarray(w_kv_a, dtype=np.float32)
    kv_a_ln_w = np.asarray(kv_a_ln_w, dtype=np.float32)
    w_kv_b = np.asarray(w_kv_b, dtype=np.float32)
    w_o = np.asarray(w_o, dtype=np.float32)

    bf = lambda a: np.ascontiguousarray(a.astype(BFNP))

    # fold q_a layernorm weight into w_q_b rows; reorder cols [nope|rope]
    wqb_eff = w_q_b * q_a_ln_w[:, None]
    wqb3 = wqb_eff.reshape(QLR, H, DN + DR)
    wqb_r = np.concatenate(
        [wqb3[:, :, :DN].reshape(QLR, H * DN),
         wqb3[:, :, DN:].reshape(QLR, H * DR)], axis=1)

    # fold kv layernorm weight into w_uk / w_uv
    wkv3 = w_kv_b.reshape(R, H, DN + DV) * kv_a_ln_w[:, None, None]
    cos_rm, sin_rm = _rope_rm()
    cos16 = np.tile(cos_rm, (1, H))
    sin16 = np.tile(sin_rm, (1, H))
    masks = _masks().reshape(4 * 128, 512)

    wqa_b = bf(w_q_a)
    wkva_b = bf(w_kv_a)
    wqb_b = bf(wqb_r)
    wo_b = bf(w_o)
    ones_b = np.ones((128, 128), dtype=BFNP)
    ident_b = np.eye(128, dtype=BFNP)

    def pm(a, p=128):
        # [n*128, m] row-chunked -> partition-major [128, n*m]
        n = a.shape[0] // p
        return np.ascontiguousarray(
            a.reshape(n, p, a.shape[1]).transpose(1, 0, 2).reshape(p, -1))

    masks_pm = pm(masks)
    in_maps = []
    for c in range(NCORES):
        h0 = HPC * c
        sl = slice(c * SL, (c + 1) * SL)
        in_maps.append({
            "xT_pm": pm(bf(x[0, sl, :].T)),
            "w_qa": wqa_b,
            "w_kva_pm": pm(wkva_b),
            "w_qb": wqb_b,
            "w_uk_pm": pm(bf(np.concatenate(
                [wkv3[:, h0, :DN], wkv3[:, h0 + 1, :DN]], axis=1))),
            "w_uv_pm": pm(bf(np.concatenate(
                [wkv3[:, h0, DN:], wkv3[:, h0 + 1, DN:]], axis=1))),
            "w_o_f": wo_b,
            "cosq_pm": pm(bf(cos_rm[sl])),
            "sinq_pm": pm(bf(sin_rm[sl])),
            "cos16_pm": pm(bf(cos16[sl])),
            "sin16_pm": pm(bf(sin16[sl])),
            "masks_pm": masks_pm,
            "ones_in": ones_b,
            "ident_in": ident_b,
        })
    return in_maps


def kernel(**inputs):
    in_maps = _make_in_maps(**inputs)
    nc = _get_nc()
    # The axon terminal occasionally reports NRT_EXEC_UNIT_UNRECOVERABLE on the
    # first load after a prior session died; a retry recovers it.
    last_exc = None
    for _ in range(3):
        try:
            res = run_bass_kernel_spmd(nc, in_maps, core_ids=list(range(NCORES)))
            break
        except Exception as e:  # noqa: BLE001
            last_exc = e
    else:
        raise last_exc
    y = np.concatenate([res.results[c]["y_sl"] for c in range(NCORES)], axis=0)
    return y.reshape(B, S, E).astype(np.float32)


if __name__ == "__main__":
    nc = _build()
    print("built ok")
